# revision 6
# baseline (speedup 1.0000x reference)
"""CFConv (SchNet continuous-filter conv) Bass kernel for 8 Trainium2 NeuronCores.

Strategy (graph/data parallel): nodes partitioned 12500/core; edges routed to
the owner of their destination node so the scatter-add is local to a core.

- Phase A: each core computes xw = x @ W1 + b1 for ALL nodes into a row-major
  DRAM table [100352, 128] (4 quarters of 25088 rows; node n at row n).
- Phase B: edges grouped by (dst superblock of 512 nodes, table quarter),
  padded to 128-edge tiles. Per-edge xw rows fetched with the Q7
  dma_gather custom op (int16 indices relative to the quarter base), up to
  8 tiles (1024 rows) per op, alternating between 2 SWDGE queues.
- Filter MLP per 4-tile window: h1 = Wf1.T @ rbfT (K=16) into PSUM, Silu+bf1
  on ACT, filt = h1_slice.T @ Wf2 per tile, msg = (filt + bf2) * xw_g on DVE.
- Scatter-add as matmul: outT[f, n] += msgm[e, f].T @ S[e, n] with S a one-hot
  [128, 512] built by is_equal(iota512, rl); one PSUM bank per superblock,
  5 superblocks live per PSUM group.
- Per superblock: final = outT.T @ W2 + b2 (4 x 128-chunk matmuls), DMA'd out.

SPMD: one program for all 8 cores; per-(core, sb, quarter) edge counts padded
to a common tile count (max over cores) so instruction streams match.
"""
import sys
sys.path.insert(0, '/opt/trn_rl_repo')
from contextlib import ExitStack

import numpy as np

import concourse.bass as bass
import concourse.bacc as bacc
import concourse.tile as tile
from concourse import library_config, mybir
from concourse.bass_utils import run_bass_kernel_spmd

N_NODES = 100000
N_EDGES = 640000
D = 128
RBF = 16
NCORES = 8
NPC = N_NODES // NCORES        # 12500 nodes per core
QR = 25088                     # table rows per quarter (int16-safe, 196 tiles)
NQ = 4
NPAD = NQ * QR                 # 100352
NTX = NPAD // 128              # 784 node tiles
SB = 512                       # dst superblock (scatter matmul free dim)
NSB = (NPC + SB - 1) // SB     # 25 superblocks per core
GRPSB = 5                      # superblocks per PSUM group (5 banks)
NGRP = NSB // GRPSB            # 5 groups
OPT = 8                        # max tiles per gather op (1024 idx wedge limit)
GRP = 4                        # edge tiles per filter-MLP window
XCH = 32                       # x tiles per phase-A chunk

F32 = mybir.dt.float32
I16 = mybir.dt.int16


def _preprocess(edge_index, edge_rbf):
    """Route edges by dest owner; group by (dst superblock, table quarter);
    pad each group to whole 128-edge tiles with a common count across cores."""
    row = np.asarray(edge_index[0], dtype=np.int64)
    col = np.asarray(edge_index[1], dtype=np.int64)
    rbf = np.asarray(edge_rbf, dtype=np.float32)

    core = row // NPC
    per_core = []
    counts = np.zeros((NCORES, NSB, NQ), dtype=np.int64)
    for c in range(NCORES):
        m = core == c
        dst = row[m] - c * NPC
        cl = col[m]
        sb = dst // SB
        rl = dst % SB
        q = cl // QR
        tr = cl % QR
        order = np.lexsort((q, sb))
        sb, rl, q, tr = sb[order], rl[order], q[order], tr[order]
        rb = rbf[m][order]
        np.add.at(counts[c], (sb, q), 1)
        per_core.append((sb, rl, q, tr, rb))

    tpbq = (counts.max(axis=0) + 127) // 128              # [NSB, NQ]
    for s in range(NSB):
        if tpbq[s].sum() == 0:
            tpbq[s][0] = 1  # keep >=1 tile so outT is always written

    # tile stream: for grp, for q, for sb in grp: tpbq[sb, q] tiles.
    # gather ops span consecutive tiles within one (grp, q) segment.
    tiles = []        # sb id per tile
    ops = []          # (t0, ntiles, queue) per gather op
    op_quarter = []   # table quarter per op
    seg_start = {}    # (sb, q) -> first tile index of its segment
    for g in range(NGRP):
        sbs = range(g * GRPSB, min((g + 1) * GRPSB, NSB))
        for q in range(NQ):
            seg0 = len(tiles)
            for s in sbs:
                seg_start[(s, q)] = len(tiles)
                tiles.extend([s] * int(tpbq[s][q]))
            nt = len(tiles) - seg0
            t0 = seg0
            while nt > 0:
                k = min(OPT, nt)
                ops.append((t0, k, len(ops) % 2))
                op_quarter.append(q)
                t0 += k
                nt -= k
    T = len(tiles)

    sb_first = {}
    sb_last = {}
    for t, s in enumerate(tiles):
        sb_first.setdefault(s, t)
        sb_last[s] = t

    cores = []
    for c in range(NCORES):
        sb, rl, q, tr, rb = per_core[c]
        idx16 = np.zeros((T * 128,), dtype=np.int16)
        rla = np.full((T * 128,), 999.0, dtype=np.float32)
        rbfT = np.zeros((RBF, T * 128), dtype=np.float32)
        e0 = 0
        # edges are sorted by (sb, q); walk groups in the same order the
        # tile stream was laid out per (sb, q) segment
        for ss in range(NSB):
            for qq in range(NQ):
                n = int(counts[c, ss, qq])
                if n == 0:
                    continue
                d0 = seg_start[(ss, qq)] * 128
                idx16[d0:d0 + n] = tr[e0:e0 + n].astype(np.int16)
                rla[d0:d0 + n] = rl[e0:e0 + n].astype(np.float32)
                rbfT[:, d0:d0 + n] = rb[e0:e0 + n].T
                e0 += n
        assert e0 == len(sb)
        # idx wrapped in 16 partitions, replicated for the 8 Q7 cores
        idxw = np.tile(idx16.reshape(T * 8, 16).T, (8, 1)).copy()
        cores.append({
            "idx16": idxw,                 # [128, T*8] int16
            "rl": rla.reshape(T, 128).T.copy(),   # [128, T]
            "rbfT": rbfT,                  # [16, T*128]
        })
    meta = dict(tiles=tiles, ops=ops, op_quarter=op_quarter,
                sb_first=sb_first, sb_last=sb_last)
    return cores, tpbq, T, meta


def _build_program(T, meta, reps=1):
    tiles, ops = meta["tiles"], meta["ops"]
    sb_first, sb_last = meta["sb_first"], meta["sb_last"]

    nc = bacc.Bacc("TRN2", target_bir_lowering=False, debug=False,
                   num_devices=NCORES, num_swdge_queues=2)

    xT_in = nc.dram_tensor("xT", [128, NPAD], F32, kind="ExternalInput").ap()
    W1_in = nc.dram_tensor("W1", [D, D], F32, kind="ExternalInput").ap()
    Wf1_in = nc.dram_tensor("Wf1", [RBF, D], F32, kind="ExternalInput").ap()
    Wf2_in = nc.dram_tensor("Wf2", [D, D], F32, kind="ExternalInput").ap()
    W2_in = nc.dram_tensor("W2", [D, D], F32, kind="ExternalInput").ap()
    b1bc_in = nc.dram_tensor("b1bc", [128, D], F32, kind="ExternalInput").ap()
    bf1c_in = nc.dram_tensor("bf1c", [128, 1], F32, kind="ExternalInput").ap()
    bf2bc_in = nc.dram_tensor("bf2bc", [128, D], F32, kind="ExternalInput").ap()
    b2bc_in = nc.dram_tensor("b2bc", [128, D], F32, kind="ExternalInput").ap()
    iota_in = nc.dram_tensor("iota512", [128, SB], F32, kind="ExternalInput").ap()
    idx_in = nc.dram_tensor("idx16", [128, T * 8], I16, kind="ExternalInput").ap()
    rl_in = nc.dram_tensor("rl", [128, T], F32, kind="ExternalInput").ap()
    rbfT_in = nc.dram_tensor("rbfT", [RBF, T * 128], F32, kind="ExternalInput").ap()
    out_ext = nc.dram_tensor("out", [NPC, D], F32, kind="ExternalOutput").ap()

    # xw table, row-major: node n at row n, features contiguous
    tab_h = nc.dram_tensor("xwtab", [NPAD, D], F32)
    tab = tab_h.ap()
    # per-quarter gather views: [QR, 128] starting at row q*QR
    tab_q = [bass.AP(tab.tensor, q * QR * D, [[D, QR], [1, D]])
             for q in range(NQ)]

    nc.gpsimd.load_library(library_config.mlp)

    NCH = (NTX + XCH - 1) // XCH

    with tile.TileContext(nc) as tc:
        with ExitStack() as ctx:
            res = ctx.enter_context(tc.tile_pool(name="res", bufs=1))
            xpool = ctx.enter_context(tc.tile_pool(name="xch", bufs=2))
            spool = ctx.enter_context(tc.tile_pool(name="stage", bufs=2))
            gpool = ctx.enter_context(tc.tile_pool(name="gath", bufs=4))
            rpool = ctx.enter_context(tc.tile_pool(name="rbfp", bufs=4))
            wpool = ctx.enter_context(tc.tile_pool(name="work", bufs=3))
            hpool = ctx.enter_context(tc.tile_pool(name="hts", bufs=2))
            opool = ctx.enter_context(tc.tile_pool(name="outs", bufs=2))
            pp_mm = ctx.enter_context(tc.tile_pool(name="psmm", bufs=2, space="PSUM"))
            pp_h1 = ctx.enter_context(tc.tile_pool(name="psh1", bufs=1, space="PSUM"))
            pp_out = ctx.enter_context(tc.tile_pool(name="psout", bufs=GRPSB,
                                                    space="PSUM"))

            W1 = res.tile([D, D], F32); nc.sync.dma_start(W1[:], W1_in[:])
            Wf1 = res.tile([RBF, D], F32); nc.sync.dma_start(Wf1[:], Wf1_in[:])
            Wf2 = res.tile([D, D], F32); nc.sync.dma_start(Wf2[:], Wf2_in[:])
            W2 = res.tile([D, D], F32); nc.sync.dma_start(W2[:], W2_in[:])
            b1bc = res.tile([128, D], F32); nc.sync.dma_start(b1bc[:], b1bc_in[:])
            bf1c = res.tile([128, 1], F32); nc.sync.dma_start(bf1c[:], bf1c_in[:])
            bf2bc = res.tile([128, D], F32); nc.sync.dma_start(bf2bc[:], bf2bc_in[:])
            b2bc = res.tile([128, D], F32); nc.sync.dma_start(b2bc[:], b2bc_in[:])
            iota = res.tile([128, SB], F32); nc.sync.dma_start(iota[:], iota_in[:])
            idx_sb = res.tile([128, T * 8], I16); nc.sync.dma_start(idx_sb[:], idx_in[:])
            rl = res.tile([128, T], F32); nc.sync.dma_start(rl[:], rl_in[:])

            for _rep in range(reps):
                # ---- phase A: xw = x @ W1 + b1 for all nodes ----
                for ch in range(NCH):
                    i0 = ch * XCH
                    nt = min(XCH, NTX - i0)
                    xch = xpool.tile([128, XCH * 128], F32, tag="xch")
                    nc.sync.dma_start(xch[:, :nt * 128],
                                      xT_in[:, i0 * 128:(i0 + nt) * 128])
                    stage = spool.tile([128, XCH * 128], F32, tag="stage")
                    for i in range(nt):
                        xw_ps = pp_mm.tile([128, 128], F32, tag="mm128",
                                           space="PSUM")
                        nc.tensor.matmul(xw_ps[:],
                                         lhsT=xch[:, i * 128:(i + 1) * 128],
                                         rhs=W1[:], start=True, stop=True)
                        nc.vector.tensor_add(stage[:, i * 128:(i + 1) * 128],
                                             xw_ps[:], b1bc[:])
                    # write [nt*128 rows, 128] of the table, partition-major
                    wv = bass.AP(tab.tensor, i0 * 128 * D,
                                 [[D, 128], [128 * D, nt], [1, D]])
                    nc.sync.dma_start(wv, stage[:, :nt * 128])

                # ---- phase B ----
                outT = {}
                for opi, (t0, ntq, qnum) in enumerate(ops):
                    q = meta["op_quarter"][opi]
                    xwg = gpool.tile([128, OPT, 128], F32, tag="xwg")
                    nc.gpsimd.dma_gather(
                        out_ap=xwg[:, :ntq, :],
                        in_ap=tab_q[q],
                        idxs_ap=idx_sb[:, t0 * 8:(t0 + ntq) * 8],
                        num_idxs=ntq * 128,
                        num_idxs_reg=ntq * 128,
                        elem_size=D,
                        queue_num=qnum)
                    rbft = rpool.tile([RBF, OPT * 128], F32, tag="rbf")
                    nc.sync.dma_start(
                        rbft[:, :ntq * 128],
                        rbfT_in[:, t0 * 128:(t0 + ntq) * 128])
                    for g0 in range(0, ntq, GRP):
                        gsz = min(GRP, ntq - g0)
                        h1 = pp_h1.tile([128, GRP * 128], F32, tag="h1",
                                        space="PSUM")
                        nc.tensor.matmul(
                            h1[:, :gsz * 128], lhsT=Wf1[:],
                            rhs=rbft[:, g0 * 128:(g0 + gsz) * 128],
                            start=True, stop=True)
                        hT = hpool.tile([128, GRP * 128], F32, tag="hT")
                        nc.scalar.activation(
                            hT[:, :gsz * 128], h1[:, :gsz * 128],
                            mybir.ActivationFunctionType.Silu,
                            bias=bf1c[:, :1], scale=1.0)
                        for i in range(gsz):
                            t = t0 + g0 + i
                            s = tiles[t]
                            if s not in outT:
                                outT[s] = pp_out.tile([128, SB], F32,
                                                      tag="outT", space="PSUM",
                                                      name=f"oT{s}")
                            S = wpool.tile([128, SB], F32, tag="S")
                            nc.vector.tensor_tensor(
                                out=S[:], in0=iota[:],
                                in1=rl[:, t:t + 1].to_broadcast([128, SB]),
                                op=mybir.AluOpType.is_equal)
                            filt_ps = pp_mm.tile([128, 128], F32, tag="mm128",
                                                 space="PSUM")
                            nc.tensor.matmul(
                                filt_ps[:],
                                lhsT=hT[:, i * 128:(i + 1) * 128],
                                rhs=Wf2[:], start=True, stop=True)
                            msg = wpool.tile([128, 128], F32, tag="msg")
                            nc.vector.scalar_tensor_tensor(
                                out=msg[:], in0=filt_ps[:], scalar=1.0,
                                in1=bf2bc[:], op0=mybir.AluOpType.mult,
                                op1=mybir.AluOpType.add)
                            msgm = wpool.tile([128, 128], F32, tag="msgm")
                            nc.vector.tensor_mul(msgm[:], msg[:],
                                                 xwg[:, g0 + i, :])
                            nc.tensor.matmul(
                                outT[s][:], lhsT=msgm[:], rhs=S[:],
                                start=(t == sb_first[s]),
                                stop=(t == sb_last[s]))
                            if t == sb_last[s]:
                                # retire superblock s
                                oT = outT.pop(s)
                                outT_sb = opool.tile([128, SB], F32,
                                                     tag="outTsb")
                                nc.vector.tensor_copy(outT_sb[:], oT[:])
                                r0 = s * SB
                                rows = min(SB, NPC - r0)
                                nchunk = (rows + 127) // 128
                                for cc in range(nchunk):
                                    fin_ps = pp_mm.tile([128, 128], F32,
                                                        tag="mm128",
                                                        space="PSUM")
                                    nc.tensor.matmul(
                                        fin_ps[:],
                                        lhsT=outT_sb[:, cc * 128:(cc + 1) * 128],
                                        rhs=W2[:], start=True, stop=True)
                                    fin = opool.tile([128, 128], F32,
                                                     tag="fin_sb")
                                    nc.vector.tensor_add(fin[:], fin_ps[:],
                                                         b2bc[:])
                                    rr = min(128, rows - cc * 128)
                                    nc.sync.dma_start(
                                        out_ext[r0 + cc * 128:
                                                r0 + cc * 128 + rr, :],
                                        fin[:rr, :])
    nc.compile()
    return nc


def _make_in_maps(x, edge_index, edge_rbf, W1, b1, Wf1, bf1, Wf2, bf2, W2, b2,
                  cores, T):
    xT = np.zeros((128, NPAD), dtype=np.float32)
    xp = np.zeros((NPAD, D), dtype=np.float32)
    xp[:N_NODES] = np.asarray(x, dtype=np.float32)
    # xT[:, i*128:(i+1)*128] is node-tile i, feature-on-partition
    xT[:] = xp.reshape(NTX, 128, D).transpose(2, 0, 1).reshape(D, NPAD)

    common = {
        "xT": xT,
        "W1": np.asarray(W1, np.float32),
        "Wf1": np.asarray(Wf1, np.float32),
        "Wf2": np.asarray(Wf2, np.float32),
        "W2": np.asarray(W2, np.float32),
        "b1bc": np.broadcast_to(np.asarray(b1, np.float32), (128, D)).copy(),
        "bf1c": np.asarray(bf1, np.float32).reshape(128, 1).copy(),
        "bf2bc": np.broadcast_to(np.asarray(bf2, np.float32), (128, D)).copy(),
        "b2bc": np.broadcast_to(np.asarray(b2, np.float32), (128, D)).copy(),
        "iota512": np.broadcast_to(np.arange(SB, dtype=np.float32),
                                   (128, SB)).copy(),
    }
    in_maps = []
    for c in range(NCORES):
        m = dict(common)
        m["idx16"] = cores[c]["idx16"]
        m["rl"] = cores[c]["rl"]
        m["rbfT"] = cores[c]["rbfT"]
        in_maps.append(m)
    return in_maps


_CACHE = {}


def kernel(x, edge_index, edge_rbf, W1, b1, Wf1, bf1, Wf2, bf2, W2, b2):
    cores, tpbq, T, meta = _preprocess(edge_index, edge_rbf)
    key = (T, tuple(np.asarray(tpbq).ravel().tolist()))
    if key not in _CACHE:
        _CACHE[key] = _build_program(T, meta, reps=1)
    nc = _CACHE[key]
    in_maps = _make_in_maps(x, edge_index, edge_rbf, W1, b1, Wf1, bf1, Wf2,
                            bf2, W2, b2, cores, T)
    res = run_bass_kernel_spmd(nc, in_maps, list(range(NCORES)))
    out = np.concatenate([res.results[c]["out"] for c in range(NCORES)],
                         axis=0)
    return out.astype(np.float32)


# revision 12
# speedup vs baseline: 1.8520x; 1.8520x over previous
"""CFConv (SchNet continuous-filter conv) Bass kernel for 8 Trainium2 NeuronCores.

Strategy (graph/data parallel): nodes partitioned 12500/core; edges routed to
the owner of their destination node so the scatter-add is local to a core.

- Phase A: each core computes xw = x @ W1 + b1 for ALL nodes into a row-major
  DRAM table [100352, 128] (4 quarters of 25088 rows; node n at row n).
- Phase B: edges grouped by (dst superblock of 512 nodes, table quarter),
  padded to 128-edge tiles. Per-edge xw rows fetched with the Q7
  dma_gather custom op (int16 indices relative to the quarter base), up to
  8 tiles (1024 rows) per op, alternating between 2 SWDGE queues.
- Filter MLP per 4-tile window: h1 = Wf1.T @ rbfT (K=16) into PSUM, Silu+bf1
  on ACT, filt = h1_slice.T @ Wf2 per tile, msg = (filt + bf2) * xw_g on DVE.
- Scatter-add as matmul: outT[f, n] += msgm[e, f].T @ S[e, n] with S a one-hot
  [128, 512] built by is_equal(iota512, rl); one PSUM bank per superblock,
  5 superblocks live per PSUM group.
- Per superblock: final = outT.T @ W2 + b2 (4 x 128-chunk matmuls), DMA'd out.

SPMD: one program for all 8 cores; per-(core, sb, quarter) edge counts padded
to a common tile count (max over cores) so instruction streams match.
"""
import sys
sys.path.insert(0, '/opt/trn_rl_repo')
from contextlib import ExitStack

import numpy as np

import concourse.bass as bass
import concourse.bacc as bacc
import concourse.tile as tile
from concourse import library_config, mybir
from concourse.bass_utils import run_bass_kernel_spmd

N_NODES = 100000
N_EDGES = 640000
D = 128
RBF = 16
NCORES = 8
NPC = N_NODES // NCORES        # 12500 nodes per core
QR = 25088                     # table rows per quarter (int16-safe, 196 tiles)
NQ = 4
NPAD = NQ * QR                 # 100352
NTX = NPAD // 128              # 784 node tiles
SB = 512                       # dst superblock (scatter matmul free dim)
NSB = (NPC + SB - 1) // SB     # 25 superblocks per core
GRPSB = 5                      # superblocks per PSUM group (5 banks)
NGRP = NSB // GRPSB            # 5 groups
OPT = 8                        # max tiles per gather op (1024 idx wedge limit)
GRP = 4                        # edge tiles per filter-MLP window
XCH = 32                       # x tiles per phase-A chunk

F32 = mybir.dt.float32
I16 = mybir.dt.int16


def _preprocess(edge_index, edge_rbf):
    """Route edges by dest owner; group by (dst superblock, table quarter);
    pad each group to whole 128-edge tiles with a common count across cores."""
    row = np.asarray(edge_index[0], dtype=np.int64)
    col = np.asarray(edge_index[1], dtype=np.int64)
    rbf = np.asarray(edge_rbf, dtype=np.float32)

    core = row // NPC
    per_core = []
    counts = np.zeros((NCORES, NSB, NQ), dtype=np.int64)
    for c in range(NCORES):
        m = core == c
        dst = row[m] - c * NPC
        cl = col[m]
        sb = dst // SB
        rl = dst % SB
        q = cl // QR
        tr = cl % QR
        order = np.lexsort((q, sb))
        sb, rl, q, tr = sb[order], rl[order], q[order], tr[order]
        rb = rbf[m][order]
        np.add.at(counts[c], (sb, q), 1)
        per_core.append((sb, rl, q, tr, rb))

    tpbq = (counts.max(axis=0) + 127) // 128              # [NSB, NQ]
    for s in range(NSB):
        if tpbq[s].sum() == 0:
            tpbq[s][0] = 1  # keep >=1 tile so outT is always written

    # tile stream: for grp, for q, for sb in grp: tpbq[sb, q] tiles.
    # gather ops span consecutive tiles within one (grp, q) segment.
    tiles = []        # sb id per tile
    ops = []          # (t0, ntiles, queue) per gather op
    op_quarter = []   # table quarter per op
    seg_start = {}    # (sb, q) -> first tile index of its segment
    for g in range(NGRP):
        sbs = range(g * GRPSB, min((g + 1) * GRPSB, NSB))
        for q in range(NQ):
            seg0 = len(tiles)
            for s in sbs:
                seg_start[(s, q)] = len(tiles)
                tiles.extend([s] * int(tpbq[s][q]))
            nt = len(tiles) - seg0
            t0 = seg0
            while nt > 0:
                k = min(OPT, nt)
                ops.append((t0, k, len(ops) % 2))
                op_quarter.append(q)
                t0 += k
                nt -= k
    T = len(tiles)

    sb_first = {}
    sb_last = {}
    for t, s in enumerate(tiles):
        sb_first.setdefault(s, t)
        sb_last[s] = t

    cores = []
    for c in range(NCORES):
        sb, rl, q, tr, rb = per_core[c]
        idx16 = np.zeros((T * 128,), dtype=np.int16)
        rla = np.full((T * 128,), 999.0, dtype=np.float32)
        rbfT = np.zeros((RBF, T * 128), dtype=np.float32)
        e0 = 0
        # edges are sorted by (sb, q); walk groups in the same order the
        # tile stream was laid out per (sb, q) segment
        for ss in range(NSB):
            for qq in range(NQ):
                n = int(counts[c, ss, qq])
                if n == 0:
                    continue
                d0 = seg_start[(ss, qq)] * 128
                idx16[d0:d0 + n] = tr[e0:e0 + n].astype(np.int16)
                rla[d0:d0 + n] = rl[e0:e0 + n].astype(np.float32)
                rbfT[:, d0:d0 + n] = rb[e0:e0 + n].T
                e0 += n
        assert e0 == len(sb)
        # idx wrapped in 16 partitions, replicated for the 8 Q7 cores
        idxw = np.tile(idx16.reshape(T * 8, 16).T, (8, 1)).copy()
        cores.append({
            "idx16": idxw,                 # [128, T*8] int16
            "rl": rla.reshape(T, 128).T.copy(),   # [128, T]
            "rbfT": rbfT,                  # [16, T*128]
        })
    meta = dict(tiles=tiles, ops=ops, op_quarter=op_quarter,
                sb_first=sb_first, sb_last=sb_last)
    return cores, tpbq, T, meta


def _build_program(T, meta, reps=1, do_gather=True, qmode="alt",
                   do_phase_a=True, do_phase_b=True, do_compute=True,
                   do_scatter=True):
    tiles, ops = meta["tiles"], meta["ops"]
    sb_first, sb_last = meta["sb_first"], meta["sb_last"]

    nc = bacc.Bacc("TRN2", target_bir_lowering=False, debug=False,
                   num_devices=NCORES, num_swdge_queues=2)

    xT_in = nc.dram_tensor("xT", [128, NPAD], F32, kind="ExternalInput").ap()
    W1_in = nc.dram_tensor("W1", [D, D], F32, kind="ExternalInput").ap()
    Wf1_in = nc.dram_tensor("Wf1", [RBF, D], F32, kind="ExternalInput").ap()
    Wf2_in = nc.dram_tensor("Wf2", [D, D], F32, kind="ExternalInput").ap()
    W2_in = nc.dram_tensor("W2", [D, D], F32, kind="ExternalInput").ap()
    b1bc_in = nc.dram_tensor("b1bc", [128, D], F32, kind="ExternalInput").ap()
    bf1c_in = nc.dram_tensor("bf1c", [128, 1], F32, kind="ExternalInput").ap()
    bf2bc_in = nc.dram_tensor("bf2bc", [128, GRP * D], F32, kind="ExternalInput").ap()
    b2bc_in = nc.dram_tensor("b2bc", [128, D], F32, kind="ExternalInput").ap()
    iota_in = nc.dram_tensor("iota512", [128, SB], F32, kind="ExternalInput").ap()
    idx_in = nc.dram_tensor("idx16", [128, T * 8], I16, kind="ExternalInput").ap()
    rl_in = nc.dram_tensor("rl", [128, T], F32, kind="ExternalInput").ap()
    rbfT_in = nc.dram_tensor("rbfT", [RBF, T * 128], F32, kind="ExternalInput").ap()
    out_ext = nc.dram_tensor("out", [NPC, D], F32, kind="ExternalOutput").ap()

    # xw table, row-major: node n at row n, features contiguous
    tab_h = nc.dram_tensor("xwtab", [NPAD, D], F32)
    tab = tab_h.ap()
    # per-quarter gather views: [QR, 128] starting at row q*QR
    tab_q = [bass.AP(tab.tensor, q * QR * D, [[D, QR], [1, D]])
             for q in range(NQ)]

    nc.gpsimd.load_library(library_config.mlp)

    NCH = (NTX + XCH - 1) // XCH

    with tile.TileContext(nc) as tc:
        with ExitStack() as ctx:
            res = ctx.enter_context(tc.tile_pool(name="res", bufs=1))
            xpool = ctx.enter_context(tc.tile_pool(name="xch", bufs=2))
            spool = ctx.enter_context(tc.tile_pool(name="stage", bufs=2))
            gpool = ctx.enter_context(tc.tile_pool(name="gath", bufs=4))
            rpool = ctx.enter_context(tc.tile_pool(name="rbfp", bufs=4))
            wpool = ctx.enter_context(tc.tile_pool(name="work", bufs=3))
            sspool = ctx.enter_context(tc.tile_pool(name="spool", bufs=2))
            hpool = ctx.enter_context(tc.tile_pool(name="hts", bufs=2))
            opool = ctx.enter_context(tc.tile_pool(name="outs", bufs=2))
            pp_mm = ctx.enter_context(tc.tile_pool(name="psmm", bufs=1, space="PSUM"))
            pp_h1 = ctx.enter_context(tc.tile_pool(name="psh1", bufs=1, space="PSUM"))
            pp_out = ctx.enter_context(tc.tile_pool(name="psout", bufs=GRPSB,
                                                    space="PSUM"))

            W1 = res.tile([D, D], F32); nc.sync.dma_start(W1[:], W1_in[:])
            Wf1 = res.tile([RBF, D], F32); nc.sync.dma_start(Wf1[:], Wf1_in[:])
            Wf2 = res.tile([D, D], F32); nc.sync.dma_start(Wf2[:], Wf2_in[:])
            W2 = res.tile([D, D], F32); nc.sync.dma_start(W2[:], W2_in[:])
            b1bc = res.tile([128, D], F32); nc.sync.dma_start(b1bc[:], b1bc_in[:])
            bf1c = res.tile([128, 1], F32); nc.sync.dma_start(bf1c[:], bf1c_in[:])
            bf2bc = res.tile([128, GRP * D], F32); nc.sync.dma_start(bf2bc[:], bf2bc_in[:])
            b2bc = res.tile([128, D], F32); nc.sync.dma_start(b2bc[:], b2bc_in[:])
            iota = res.tile([128, SB], F32); nc.sync.dma_start(iota[:], iota_in[:])
            idx_sb = res.tile([128, T * 8], I16); nc.sync.dma_start(idx_sb[:], idx_in[:])
            rl = res.tile([128, T], F32); nc.sync.dma_start(rl[:], rl_in[:])

            for _rep in range(reps):
                # ---- phase A: xw = x @ W1 + b1 for all nodes ----
                for ch in range(NCH if do_phase_a else 0):
                    i0 = ch * XCH
                    nt = min(XCH, NTX - i0)
                    xch = xpool.tile([128, XCH * 128], F32, tag="xch")
                    nc.sync.dma_start(xch[:, :nt * 128],
                                      xT_in[:, i0 * 128:(i0 + nt) * 128])
                    stage = spool.tile([128, XCH * 128], F32, tag="stage")
                    for i in range(nt):
                        xw_ps = pp_mm.tile([128, 128], F32, tag="mm128",
                                           space="PSUM")
                        nc.tensor.matmul(xw_ps[:],
                                         lhsT=xch[:, i * 128:(i + 1) * 128],
                                         rhs=W1[:], start=True, stop=True)
                        nc.vector.tensor_add(stage[:, i * 128:(i + 1) * 128],
                                             xw_ps[:], b1bc[:])
                    # write [nt*128 rows, 128] of the table, partition-major
                    wv = bass.AP(tab.tensor, i0 * 128 * D,
                                 [[D, 128], [128 * D, nt], [1, D]])
                    nc.sync.dma_start(wv, stage[:, :nt * 128])

                # ---- phase B ----
                outT = {}
                for opi, (t0, ntq, qnum) in enumerate(ops if do_phase_b else []):
                    q = meta["op_quarter"][opi]
                    xwg = gpool.tile([128, OPT, 128], F32, tag="xwg")
                    if do_gather:
                        nc.gpsimd.dma_gather(
                            out_ap=xwg[:, :ntq, :],
                            in_ap=tab_q[q],
                            idxs_ap=idx_sb[:, t0 * 8:(t0 + ntq) * 8],
                            num_idxs=ntq * 128,
                            num_idxs_reg=ntq * 128,
                            elem_size=D,
                            queue_num=(qnum if qmode == "alt" else 0))
                    else:
                        nc.vector.memset(xwg[:, :ntq, :], 1.0)
                    rbft = rpool.tile([RBF, OPT * 128], F32, tag="rbf")
                    nc.sync.dma_start(
                        rbft[:, :ntq * 128],
                        rbfT_in[:, t0 * 128:(t0 + ntq) * 128])
                    S3 = sspool.tile([128, OPT, SB], F32, tag="S3")
                    iota_a = iota[:]
                    iota_v = bass.AP(iota_a.tensor, iota_a.offset,
                                     [iota_a.ap[0], [0, ntq], [1, SB]])
                    rl_s = rl[:, t0:t0 + ntq]
                    rl_v = bass.AP(rl_s.tensor, rl_s.offset,
                                   [rl_s.ap[0], [1, ntq], [0, SB]])
                    nc.vector.tensor_tensor(
                        out=S3[:, :ntq, :], in0=iota_v, in1=rl_v,
                        op=mybir.AluOpType.is_equal)
                    if not do_compute:
                        continue
                    for g0 in range(0, ntq, GRP):
                        gsz = min(GRP, ntq - g0)
                        h1 = pp_h1.tile([128, GRP * 128], F32, tag="h1",
                                        space="PSUM")
                        nc.tensor.matmul(
                            h1[:, :gsz * 128], lhsT=Wf1[:],
                            rhs=rbft[:, g0 * 128:(g0 + gsz) * 128],
                            start=True, stop=True)
                        hT = hpool.tile([128, GRP * 128], F32, tag="hT")
                        nc.scalar.activation(
                            hT[:, :gsz * 128], h1[:, :gsz * 128],
                            mybir.ActivationFunctionType.Silu,
                            bias=bf1c[:, :1], scale=1.0)
                        filt_ps = pp_h1.tile([128, GRP * 128], F32,
                                             tag="filtps", space="PSUM")
                        for i in range(gsz):
                            nc.tensor.matmul(
                                filt_ps[:, i * 128:(i + 1) * 128],
                                lhsT=hT[:, i * 128:(i + 1) * 128],
                                rhs=Wf2[:], start=True, stop=True)
                        msg = wpool.tile([128, GRP * 128], F32, tag="msg")
                        nc.vector.scalar_tensor_tensor(
                            out=msg[:, :gsz * 128], in0=filt_ps[:, :gsz * 128],
                            scalar=1.0, in1=bf2bc[:, :gsz * 128],
                            op0=mybir.AluOpType.mult,
                            op1=mybir.AluOpType.add)
                        msgm = wpool.tile([128, GRP, 128], F32, tag="msgm")
                        nc.vector.tensor_mul(msgm[:, :gsz, :],
                                             msg[:, :gsz * 128],
                                             xwg[:, g0:g0 + gsz, :])
                        for i in range(gsz):
                            t = t0 + g0 + i
                            s = tiles[t]
                            if do_scatter and s not in outT:
                                outT[s] = pp_out.tile([128, SB], F32,
                                                      tag="outT", space="PSUM",
                                                      name=f"oT{s}")
                            if do_scatter:
                                nc.tensor.matmul(
                                    outT[s][:], lhsT=msgm[:, i, :],
                                    rhs=S3[:, g0 + i, :],
                                    start=(t == sb_first[s]),
                                    stop=(t == sb_last[s]))
                            if do_scatter and t == sb_last[s]:
                                # retire superblock s
                                oT = outT.pop(s)
                                outT_sb = opool.tile([128, SB], F32,
                                                     tag="outTsb")
                                nc.vector.tensor_copy(outT_sb[:], oT[:])
                                r0 = s * SB
                                rows = min(SB, NPC - r0)
                                nchunk = (rows + 127) // 128
                                for cc in range(nchunk):
                                    fin_ps = pp_mm.tile([128, 128], F32,
                                                        tag="mm128",
                                                        space="PSUM")
                                    nc.tensor.matmul(
                                        fin_ps[:],
                                        lhsT=outT_sb[:, cc * 128:(cc + 1) * 128],
                                        rhs=W2[:], start=True, stop=True)
                                    fin = opool.tile([128, 128], F32,
                                                     tag="fin_sb")
                                    nc.vector.tensor_add(fin[:], fin_ps[:],
                                                         b2bc[:])
                                    rr = min(128, rows - cc * 128)
                                    nc.sync.dma_start(
                                        out_ext[r0 + cc * 128:
                                                r0 + cc * 128 + rr, :],
                                        fin[:rr, :])
    nc.compile()
    return nc


def _make_in_maps(x, edge_index, edge_rbf, W1, b1, Wf1, bf1, Wf2, bf2, W2, b2,
                  cores, T):
    xT = np.zeros((128, NPAD), dtype=np.float32)
    xp = np.zeros((NPAD, D), dtype=np.float32)
    xp[:N_NODES] = np.asarray(x, dtype=np.float32)
    # xT[:, i*128:(i+1)*128] is node-tile i, feature-on-partition
    xT[:] = xp.reshape(NTX, 128, D).transpose(2, 0, 1).reshape(D, NPAD)

    common = {
        "xT": xT,
        "W1": np.asarray(W1, np.float32),
        "Wf1": np.asarray(Wf1, np.float32),
        "Wf2": np.asarray(Wf2, np.float32),
        "W2": np.asarray(W2, np.float32),
        "b1bc": np.broadcast_to(np.asarray(b1, np.float32), (128, D)).copy(),
        "bf1c": np.asarray(bf1, np.float32).reshape(128, 1).copy(),
        "bf2bc": np.tile(np.broadcast_to(np.asarray(bf2, np.float32), (128, D)), (1, GRP)).copy(),
        "b2bc": np.broadcast_to(np.asarray(b2, np.float32), (128, D)).copy(),
        "iota512": np.broadcast_to(np.arange(SB, dtype=np.float32),
                                   (128, SB)).copy(),
    }
    in_maps = []
    for c in range(NCORES):
        m = dict(common)
        m["idx16"] = cores[c]["idx16"]
        m["rl"] = cores[c]["rl"]
        m["rbfT"] = cores[c]["rbfT"]
        in_maps.append(m)
    return in_maps


_CACHE = {}


def kernel(x, edge_index, edge_rbf, W1, b1, Wf1, bf1, Wf2, bf2, W2, b2):
    cores, tpbq, T, meta = _preprocess(edge_index, edge_rbf)
    key = (T, tuple(np.asarray(tpbq).ravel().tolist()))
    if key not in _CACHE:
        _CACHE[key] = _build_program(T, meta, reps=1)
    nc = _CACHE[key]
    in_maps = _make_in_maps(x, edge_index, edge_rbf, W1, b1, Wf1, bf1, Wf2,
                            bf2, W2, b2, cores, T)
    res = run_bass_kernel_spmd(nc, in_maps, list(range(NCORES)))
    out = np.concatenate([res.results[c]["out"] for c in range(NCORES)],
                         axis=0)
    return out.astype(np.float32)


# revision 14
# speedup vs baseline: 6.8201x; 3.6827x over previous
"""CFConv (SchNet continuous-filter conv) Bass kernel for 8 Trainium2 NeuronCores.

Strategy (graph/data parallel): nodes partitioned 12500/core; edges routed to
the owner of their destination node so the scatter-add is local to a core.

- Phase A: each core computes xw = x @ W1 + b1 for ALL nodes into a row-major
  DRAM table [100352, 128] (4 quarters of 25088 rows; node n at row n).
- Phase B: edges grouped by (dst superblock of 512 nodes, table quarter),
  padded to 128-edge tiles. Per-edge xw rows fetched with the Q7
  dma_gather custom op (int16 indices relative to the quarter base), up to
  8 tiles (1024 rows) per op, alternating between 2 SWDGE queues.
- Filter MLP per 4-tile window: h1 = Wf1.T @ rbfT (K=16) into PSUM, Silu+bf1
  on ACT, filt = h1_slice.T @ Wf2 per tile, msg = (filt + bf2) * xw_g on DVE.
- Scatter-add as matmul: outT[f, n] += msgm[e, f].T @ S[e, n] with S a one-hot
  [128, 512] built by is_equal(iota512, rl); one PSUM bank per superblock,
  5 superblocks live per PSUM group.
- Per superblock: final = outT.T @ W2 + b2 (4 x 128-chunk matmuls), DMA'd out.

SPMD: one program for all 8 cores; per-(core, sb, quarter) edge counts padded
to a common tile count (max over cores) so instruction streams match.
"""
import sys
sys.path.insert(0, '/opt/trn_rl_repo')
from contextlib import ExitStack

import numpy as np

import concourse.bass as bass
import concourse.bacc as bacc
import concourse.tile as tile
from concourse import library_config, mybir
from concourse.bass_utils import run_bass_kernel_spmd

N_NODES = 100000
N_EDGES = 640000
D = 128
RBF = 16
NCORES = 8
NPC = N_NODES // NCORES        # 12500 nodes per core
QR = 25088                     # table rows per quarter (int16-safe, 196 tiles)
NQ = 4
NPAD = NQ * QR                 # 100352
NTX = NPAD // 128              # 784 node tiles
SB = 512                       # dst superblock (scatter matmul free dim)
NSB = (NPC + SB - 1) // SB     # 25 superblocks per core
GRPSB = 5                      # superblocks per PSUM group (5 banks)
NGRP = NSB // GRPSB            # 5 groups
OPT = 8                        # max tiles per gather op (1024 idx wedge limit)
GRP = 4                        # edge tiles per filter-MLP window
XCH = 32                       # x tiles per phase-A chunk

F32 = mybir.dt.float32
I16 = mybir.dt.int16


def _preprocess(edge_index, edge_rbf):
    """Route edges by dest owner; group by (dst superblock, table quarter);
    pad each group to whole 128-edge tiles with a common count across cores."""
    row = np.asarray(edge_index[0], dtype=np.int64)
    col = np.asarray(edge_index[1], dtype=np.int64)
    rbf = np.asarray(edge_rbf, dtype=np.float32)

    core = row // NPC
    per_core = []
    counts = np.zeros((NCORES, NSB, NQ), dtype=np.int64)
    for c in range(NCORES):
        m = core == c
        dst = row[m] - c * NPC
        cl = col[m]
        sb = dst // SB
        rl = dst % SB
        q = cl // QR
        tr = cl % QR
        order = np.lexsort((q, sb))
        sb, rl, q, tr = sb[order], rl[order], q[order], tr[order]
        rb = rbf[m][order]
        np.add.at(counts[c], (sb, q), 1)
        per_core.append((sb, rl, q, tr, rb))

    tpbq = (counts.max(axis=0) + 127) // 128              # [NSB, NQ]
    for s in range(NSB):
        if tpbq[s].sum() == 0:
            tpbq[s][0] = 1  # keep >=1 tile so outT is always written

    # tile stream: for grp, for q, for sb in grp: tpbq[sb, q] tiles.
    # gather ops span consecutive tiles within one (grp, q) segment.
    tiles = []        # sb id per tile
    ops = []          # (t0, ntiles, queue) per gather op
    op_quarter = []   # table quarter per op
    seg_start = {}    # (sb, q) -> first tile index of its segment
    for g in range(NGRP):
        sbs = range(g * GRPSB, min((g + 1) * GRPSB, NSB))
        for q in range(NQ):
            seg0 = len(tiles)
            for s in sbs:
                seg_start[(s, q)] = len(tiles)
                tiles.extend([s] * int(tpbq[s][q]))
            nt = len(tiles) - seg0
            t0 = seg0
            while nt > 0:
                k = min(OPT, nt)
                ops.append((t0, k, len(ops) % 2))
                op_quarter.append(q)
                t0 += k
                nt -= k
    T = len(tiles)

    sb_first = {}
    sb_last = {}
    for t, s in enumerate(tiles):
        sb_first.setdefault(s, t)
        sb_last[s] = t

    cores = []
    for c in range(NCORES):
        sb, rl, q, tr, rb = per_core[c]
        idx16 = np.zeros((T * 128,), dtype=np.int16)
        rla = np.full((T * 128,), 999.0, dtype=np.float32)
        rbfT = np.zeros((RBF, T * 128), dtype=np.float32)
        e0 = 0
        # edges are sorted by (sb, q); walk groups in the same order the
        # tile stream was laid out per (sb, q) segment
        for ss in range(NSB):
            for qq in range(NQ):
                n = int(counts[c, ss, qq])
                if n == 0:
                    continue
                d0 = seg_start[(ss, qq)] * 128
                idx16[d0:d0 + n] = tr[e0:e0 + n].astype(np.int16)
                rla[d0:d0 + n] = rl[e0:e0 + n].astype(np.float32)
                rbfT[:, d0:d0 + n] = rb[e0:e0 + n].T
                e0 += n
        assert e0 == len(sb)
        # idx wrapped in 16 partitions, replicated for the 8 Q7 cores
        idxw = np.tile(idx16.reshape(T * 8, 16).T, (8, 1)).copy()
        cores.append({
            "idx16": idxw,                 # [128, T*8] int16
            "rl": rla.reshape(T, 128).T.copy(),   # [128, T]
            "rbfT": rbfT,                  # [16, T*128]
        })
    meta = dict(tiles=tiles, ops=ops, op_quarter=op_quarter,
                sb_first=sb_first, sb_last=sb_last)
    return cores, tpbq, T, meta


def _build_program(T, meta, reps=1, do_gather=True, qmode="alt",
                   do_phase_a=True, do_phase_b=True, do_compute=True,
                   do_scatter=True):
    tiles, ops = meta["tiles"], meta["ops"]
    sb_first, sb_last = meta["sb_first"], meta["sb_last"]

    nc = bacc.Bacc("TRN2", target_bir_lowering=False, debug=False,
                   num_devices=NCORES, num_swdge_queues=2)

    xT_in = nc.dram_tensor("xT", [128, NPAD], F32, kind="ExternalInput").ap()
    W1_in = nc.dram_tensor("W1", [D, D], F32, kind="ExternalInput").ap()
    Wf1_in = nc.dram_tensor("Wf1", [RBF, D], F32, kind="ExternalInput").ap()
    Wf2_in = nc.dram_tensor("Wf2", [D, D], F32, kind="ExternalInput").ap()
    W2_in = nc.dram_tensor("W2", [D, D], F32, kind="ExternalInput").ap()
    b1bc_in = nc.dram_tensor("b1bc", [128, GRP * D], F32, kind="ExternalInput").ap()
    bf1c_in = nc.dram_tensor("bf1c", [128, 1], F32, kind="ExternalInput").ap()
    bf2bc_in = nc.dram_tensor("bf2bc", [128, GRP * D], F32, kind="ExternalInput").ap()
    b2bc_in = nc.dram_tensor("b2bc", [128, D], F32, kind="ExternalInput").ap()
    iota_in = nc.dram_tensor("iota512", [128, SB], F32, kind="ExternalInput").ap()
    idx_in = nc.dram_tensor("idx16", [128, T * 8], I16, kind="ExternalInput").ap()
    rl_in = nc.dram_tensor("rl", [128, T], F32, kind="ExternalInput").ap()
    rbfT_in = nc.dram_tensor("rbfT", [RBF, T * 128], F32, kind="ExternalInput").ap()
    out_ext = nc.dram_tensor("out", [NPC, D], F32, kind="ExternalOutput").ap()

    # xw table, row-major: node n at row n, features contiguous
    tab_h = nc.dram_tensor("xwtab", [NPAD, D], F32)
    tab = tab_h.ap()
    # per-quarter gather views: [QR, 128] starting at row q*QR
    tab_q = [bass.AP(tab.tensor, q * QR * D, [[D, QR], [1, D]])
             for q in range(NQ)]

    nc.gpsimd.load_library(library_config.mlp)

    NCH = (NTX + XCH - 1) // XCH

    with tile.TileContext(nc) as tc:
        with ExitStack() as ctx:
            res = ctx.enter_context(tc.tile_pool(name="res", bufs=1))
            xpool = ctx.enter_context(tc.tile_pool(name="xch", bufs=2))
            spool = ctx.enter_context(tc.tile_pool(name="stage", bufs=2))
            gpool = ctx.enter_context(tc.tile_pool(name="gath", bufs=4))
            rpool = ctx.enter_context(tc.tile_pool(name="rbfp", bufs=4))
            wpool = ctx.enter_context(tc.tile_pool(name="work", bufs=3))
            sspool = ctx.enter_context(tc.tile_pool(name="spool", bufs=2))
            hpool = ctx.enter_context(tc.tile_pool(name="hts", bufs=2))
            opool = ctx.enter_context(tc.tile_pool(name="outs", bufs=2))
            pp_mm = ctx.enter_context(tc.tile_pool(name="psmm", bufs=1, space="PSUM"))
            pp_h1 = ctx.enter_context(tc.tile_pool(name="psh1", bufs=1, space="PSUM"))
            pp_out = ctx.enter_context(tc.tile_pool(name="psout", bufs=GRPSB,
                                                    space="PSUM"))

            W1 = res.tile([D, D], F32); nc.sync.dma_start(W1[:], W1_in[:])
            Wf1 = res.tile([RBF, D], F32); nc.sync.dma_start(Wf1[:], Wf1_in[:])
            Wf2 = res.tile([D, D], F32); nc.sync.dma_start(Wf2[:], Wf2_in[:])
            W2 = res.tile([D, D], F32); nc.sync.dma_start(W2[:], W2_in[:])
            b1bc4 = res.tile([128, GRP * D], F32); nc.sync.dma_start(b1bc4[:], b1bc_in[:])
            bf1c = res.tile([128, 1], F32); nc.sync.dma_start(bf1c[:], bf1c_in[:])
            bf2bc = res.tile([128, GRP * D], F32); nc.sync.dma_start(bf2bc[:], bf2bc_in[:])
            b2bc = res.tile([128, D], F32); nc.sync.dma_start(b2bc[:], b2bc_in[:])
            iota = res.tile([128, SB], F32); nc.sync.dma_start(iota[:], iota_in[:])
            idx_sb = res.tile([128, T * 8], I16); nc.sync.dma_start(idx_sb[:], idx_in[:])
            rl = res.tile([128, T], F32); nc.sync.dma_start(rl[:], rl_in[:])

            for _rep in range(reps):
                # ---- phase A: xw = x @ W1 + b1 for all nodes ----
                for ch in range(NCH if do_phase_a else 0):
                    i0 = ch * XCH
                    nt = min(XCH, NTX - i0)
                    xch = xpool.tile([128, XCH * 128], F32, tag="xch")
                    nc.sync.dma_start(xch[:, :nt * 128],
                                      xT_in[:, i0 * 128:(i0 + nt) * 128])
                    stage = spool.tile([128, XCH * 128], F32, tag="stage")
                    for i0g in range(0, nt, GRP):
                        gg = min(GRP, nt - i0g)
                        xw_ps = pp_h1.tile([128, GRP * 128], F32, tag="h1",
                                           space="PSUM")
                        for i in range(i0g, i0g + gg):
                            nc.tensor.matmul(
                                xw_ps[:, (i - i0g) * 128:(i - i0g + 1) * 128],
                                lhsT=xch[:, i * 128:(i + 1) * 128],
                                rhs=W1[:], start=True, stop=True)
                        nc.vector.tensor_add(
                            stage[:, i0g * 128:(i0g + gg) * 128],
                            xw_ps[:, :gg * 128], b1bc4[:, :gg * 128])
                    # write [nt*128 rows, 128] of the table, partition-major
                    wv = bass.AP(tab.tensor, i0 * 128 * D,
                                 [[D, 128], [128 * D, nt], [1, D]])
                    nc.sync.dma_start(wv, stage[:, :nt * 128])

                # ---- phase B ----
                outT = {}
                for opi, (t0, ntq, qnum) in enumerate(ops if do_phase_b else []):
                    q = meta["op_quarter"][opi]
                    xwg = gpool.tile([128, OPT, 128], F32, tag="xwg")
                    if do_gather:
                        nc.gpsimd.dma_gather(
                            out_ap=xwg[:, :ntq, :],
                            in_ap=tab_q[q],
                            idxs_ap=idx_sb[:, t0 * 8:(t0 + ntq) * 8],
                            num_idxs=ntq * 128,
                            num_idxs_reg=ntq * 128,
                            elem_size=D,
                            queue_num=(qnum if qmode == "alt" else 0))
                    else:
                        nc.vector.memset(xwg[:, :ntq, :], 1.0)
                    rbft = rpool.tile([RBF, OPT * 128], F32, tag="rbf")
                    nc.sync.dma_start(
                        rbft[:, :ntq * 128],
                        rbfT_in[:, t0 * 128:(t0 + ntq) * 128])
                    S3 = sspool.tile([128, OPT, SB], F32, tag="S3")
                    iota_a = iota[:]
                    iota_v = bass.AP(iota_a.tensor, iota_a.offset,
                                     [iota_a.ap[0], [0, ntq], [1, SB]])
                    rl_s = rl[:, t0:t0 + ntq]
                    rl_v = bass.AP(rl_s.tensor, rl_s.offset,
                                   [rl_s.ap[0], [1, ntq], [0, SB]])
                    nc.vector.tensor_tensor(
                        out=S3[:, :ntq, :], in0=iota_v, in1=rl_v,
                        op=mybir.AluOpType.is_equal)
                    if not do_compute:
                        continue
                    for g0 in range(0, ntq, GRP):
                        gsz = min(GRP, ntq - g0)
                        h1 = pp_h1.tile([128, GRP * 128], F32, tag="h1",
                                        space="PSUM")
                        nc.tensor.matmul(
                            h1[:, :gsz * 128], lhsT=Wf1[:],
                            rhs=rbft[:, g0 * 128:(g0 + gsz) * 128],
                            start=True, stop=True)
                        hT = hpool.tile([128, GRP * 128], F32, tag="hT")
                        nc.scalar.activation(
                            hT[:, :gsz * 128], h1[:, :gsz * 128],
                            mybir.ActivationFunctionType.Silu,
                            bias=bf1c[:, :1], scale=1.0)
                        filt_ps = pp_h1.tile([128, GRP * 128], F32,
                                             tag="h1", space="PSUM")
                        for i in range(gsz):
                            nc.tensor.matmul(
                                filt_ps[:, i * 128:(i + 1) * 128],
                                lhsT=hT[:, i * 128:(i + 1) * 128],
                                rhs=Wf2[:], start=True, stop=True)
                        msg = wpool.tile([128, GRP * 128], F32, tag="msg")
                        nc.vector.scalar_tensor_tensor(
                            out=msg[:, :gsz * 128], in0=filt_ps[:, :gsz * 128],
                            scalar=1.0, in1=bf2bc[:, :gsz * 128],
                            op0=mybir.AluOpType.mult,
                            op1=mybir.AluOpType.add)
                        msgm = wpool.tile([128, GRP, 128], F32, tag="msgm")
                        nc.vector.tensor_mul(msgm[:, :gsz, :],
                                             msg[:, :gsz * 128],
                                             xwg[:, g0:g0 + gsz, :])
                        for i in range(gsz):
                            t = t0 + g0 + i
                            s = tiles[t]
                            if do_scatter and s not in outT:
                                outT[s] = pp_out.tile([128, SB], F32,
                                                      tag="outT", space="PSUM",
                                                      name=f"oT{s}")
                            if do_scatter:
                                nc.tensor.matmul(
                                    outT[s][:], lhsT=msgm[:, i, :],
                                    rhs=S3[:, g0 + i, :],
                                    start=(t == sb_first[s]),
                                    stop=(t == sb_last[s]))
                            if do_scatter and t == sb_last[s]:
                                # retire superblock s
                                oT = outT.pop(s)
                                outT_sb = opool.tile([128, SB], F32,
                                                     tag="outTsb")
                                nc.vector.tensor_copy(outT_sb[:], oT[:])
                                r0 = s * SB
                                rows = min(SB, NPC - r0)
                                nchunk = (rows + 127) // 128
                                for cc in range(nchunk):
                                    fin_ps = pp_mm.tile([128, 128], F32,
                                                        tag="mm128",
                                                        space="PSUM")
                                    nc.tensor.matmul(
                                        fin_ps[:],
                                        lhsT=outT_sb[:, cc * 128:(cc + 1) * 128],
                                        rhs=W2[:], start=True, stop=True)
                                    fin = opool.tile([128, 128], F32,
                                                     tag="fin_sb")
                                    nc.vector.tensor_add(fin[:], fin_ps[:],
                                                         b2bc[:])
                                    rr = min(128, rows - cc * 128)
                                    nc.sync.dma_start(
                                        out_ext[r0 + cc * 128:
                                                r0 + cc * 128 + rr, :],
                                        fin[:rr, :])
    nc.compile()
    return nc


def _make_in_maps(x, edge_index, edge_rbf, W1, b1, Wf1, bf1, Wf2, bf2, W2, b2,
                  cores, T):
    xT = np.zeros((128, NPAD), dtype=np.float32)
    xp = np.zeros((NPAD, D), dtype=np.float32)
    xp[:N_NODES] = np.asarray(x, dtype=np.float32)
    # xT[:, i*128:(i+1)*128] is node-tile i, feature-on-partition
    xT[:] = xp.reshape(NTX, 128, D).transpose(2, 0, 1).reshape(D, NPAD)

    common = {
        "xT": xT,
        "W1": np.asarray(W1, np.float32),
        "Wf1": np.asarray(Wf1, np.float32),
        "Wf2": np.asarray(Wf2, np.float32),
        "W2": np.asarray(W2, np.float32),
        "b1bc": np.tile(np.broadcast_to(np.asarray(b1, np.float32), (128, D)), (1, GRP)).copy(),
        "bf1c": np.asarray(bf1, np.float32).reshape(128, 1).copy(),
        "bf2bc": np.tile(np.broadcast_to(np.asarray(bf2, np.float32), (128, D)), (1, GRP)).copy(),
        "b2bc": np.broadcast_to(np.asarray(b2, np.float32), (128, D)).copy(),
        "iota512": np.broadcast_to(np.arange(SB, dtype=np.float32),
                                   (128, SB)).copy(),
    }
    in_maps = []
    for c in range(NCORES):
        m = dict(common)
        m["idx16"] = cores[c]["idx16"]
        m["rl"] = cores[c]["rl"]
        m["rbfT"] = cores[c]["rbfT"]
        in_maps.append(m)
    return in_maps


_CACHE = {}


def kernel(x, edge_index, edge_rbf, W1, b1, Wf1, bf1, Wf2, bf2, W2, b2):
    cores, tpbq, T, meta = _preprocess(edge_index, edge_rbf)
    key = (T, tuple(np.asarray(tpbq).ravel().tolist()))
    if key not in _CACHE:
        _CACHE[key] = _build_program(T, meta, reps=1)
    nc = _CACHE[key]
    in_maps = _make_in_maps(x, edge_index, edge_rbf, W1, b1, Wf1, bf1, Wf2,
                            bf2, W2, b2, cores, T)
    res = run_bass_kernel_spmd(nc, in_maps, list(range(NCORES)))
    out = np.concatenate([res.results[c]["out"] for c in range(NCORES)],
                         axis=0)
    return out.astype(np.float32)


# revision 15
# speedup vs baseline: 8.2905x; 1.2156x over previous
"""CFConv (SchNet continuous-filter conv) Bass kernel for 8 Trainium2 NeuronCores.

Strategy (graph/data parallel): nodes partitioned 12500/core; edges routed to
the owner of their destination node so the scatter-add is local to a core.

- Phase A: each core computes xw = x @ W1 + b1 for ALL nodes into a row-major
  DRAM table [100352, 128] (4 quarters of 25088 rows; node n at row n).
- Phase B: edges grouped by (dst superblock of 512 nodes, table quarter),
  padded to 128-edge tiles. Per-edge xw rows fetched with the Q7
  dma_gather custom op (int16 indices relative to the quarter base), up to
  8 tiles (1024 rows) per op, alternating between 2 SWDGE queues.
- Filter MLP per 4-tile window: h1 = Wf1.T @ rbfT (K=16) into PSUM, Silu+bf1
  on ACT, filt = h1_slice.T @ Wf2 per tile, msg = (filt + bf2) * xw_g on DVE.
- Scatter-add as matmul: outT[f, n] += msgm[e, f].T @ S[e, n] with S a one-hot
  [128, 512] built by is_equal(iota512, rl); one PSUM bank per superblock,
  5 superblocks live per PSUM group.
- Per superblock: final = outT.T @ W2 + b2 (4 x 128-chunk matmuls), DMA'd out.

SPMD: one program for all 8 cores; per-(core, sb, quarter) edge counts padded
to a common tile count (max over cores) so instruction streams match.
"""
import sys
sys.path.insert(0, '/opt/trn_rl_repo')
from contextlib import ExitStack

import numpy as np

import concourse.bass as bass
import concourse.bacc as bacc
import concourse.tile as tile
from concourse import library_config, mybir
from concourse.bass_utils import run_bass_kernel_spmd

N_NODES = 100000
N_EDGES = 640000
D = 128
RBF = 16
NCORES = 8
NPC = N_NODES // NCORES        # 12500 nodes per core
QR = 25088                     # table rows per quarter (int16-safe, 196 tiles)
NQ = 4
NPAD = NQ * QR                 # 100352
NTX = NPAD // 128              # 784 node tiles
SB = 512                       # dst superblock (scatter matmul free dim)
NSB = (NPC + SB - 1) // SB     # 25 superblocks per core
GRPSB = 5                      # superblocks per PSUM group (5 banks)
NGRP = NSB // GRPSB            # 5 groups
OPT = 8                        # max tiles per gather op (1024 idx wedge limit)
GRP = 8                        # edge tiles per filter-MLP window
XCH = 32                       # x tiles per phase-A chunk

F32 = mybir.dt.float32
I16 = mybir.dt.int16


def _preprocess(edge_index, edge_rbf):
    """Route edges by dest owner; group by (dst superblock, table quarter);
    pad each group to whole 128-edge tiles with a common count across cores."""
    row = np.asarray(edge_index[0], dtype=np.int64)
    col = np.asarray(edge_index[1], dtype=np.int64)
    rbf = np.asarray(edge_rbf, dtype=np.float32)

    core = row // NPC
    per_core = []
    counts = np.zeros((NCORES, NSB, NQ), dtype=np.int64)
    for c in range(NCORES):
        m = core == c
        dst = row[m] - c * NPC
        cl = col[m]
        sb = dst // SB
        rl = dst % SB
        q = cl // QR
        tr = cl % QR
        order = np.lexsort((q, sb))
        sb, rl, q, tr = sb[order], rl[order], q[order], tr[order]
        rb = rbf[m][order]
        np.add.at(counts[c], (sb, q), 1)
        per_core.append((sb, rl, q, tr, rb))

    tpbq = (counts.max(axis=0) + 127) // 128              # [NSB, NQ]
    for s in range(NSB):
        if tpbq[s].sum() == 0:
            tpbq[s][0] = 1  # keep >=1 tile so outT is always written

    # tile stream: for grp, for q, for sb in grp: tpbq[sb, q] tiles.
    # gather ops span consecutive tiles within one (grp, q) segment.
    tiles = []        # sb id per tile
    ops = []          # (t0, ntiles, queue) per gather op
    op_quarter = []   # table quarter per op
    seg_start = {}    # (sb, q) -> first tile index of its segment
    for g in range(NGRP):
        sbs = range(g * GRPSB, min((g + 1) * GRPSB, NSB))
        for q in range(NQ):
            seg0 = len(tiles)
            for s in sbs:
                seg_start[(s, q)] = len(tiles)
                tiles.extend([s] * int(tpbq[s][q]))
            nt = len(tiles) - seg0
            t0 = seg0
            while nt > 0:
                k = min(OPT, nt)
                ops.append((t0, k, len(ops) % 2))
                op_quarter.append(q)
                t0 += k
                nt -= k
    T = len(tiles)

    sb_first = {}
    sb_last = {}
    for t, s in enumerate(tiles):
        sb_first.setdefault(s, t)
        sb_last[s] = t

    cores = []
    for c in range(NCORES):
        sb, rl, q, tr, rb = per_core[c]
        idx16 = np.zeros((T * 128,), dtype=np.int16)
        rla = np.full((T * 128,), 999.0, dtype=np.float32)
        rbfT = np.zeros((RBF, T * 128), dtype=np.float32)
        e0 = 0
        # edges are sorted by (sb, q); walk groups in the same order the
        # tile stream was laid out per (sb, q) segment
        for ss in range(NSB):
            for qq in range(NQ):
                n = int(counts[c, ss, qq])
                if n == 0:
                    continue
                d0 = seg_start[(ss, qq)] * 128
                idx16[d0:d0 + n] = tr[e0:e0 + n].astype(np.int16)
                rla[d0:d0 + n] = rl[e0:e0 + n].astype(np.float32)
                rbfT[:, d0:d0 + n] = rb[e0:e0 + n].T
                e0 += n
        assert e0 == len(sb)
        # idx wrapped in 16 partitions, replicated for the 8 Q7 cores
        idxw = np.tile(idx16.reshape(T * 8, 16).T, (8, 1)).copy()
        cores.append({
            "idx16": idxw,                 # [128, T*8] int16
            "rl": rla.reshape(T, 128).T.copy(),   # [128, T]
            "rbfT": rbfT,                  # [16, T*128]
        })
    meta = dict(tiles=tiles, ops=ops, op_quarter=op_quarter,
                sb_first=sb_first, sb_last=sb_last)
    return cores, tpbq, T, meta


def _build_program(T, meta, reps=1, do_gather=True, qmode="alt",
                   do_phase_a=True, do_phase_b=True, do_compute=True,
                   do_scatter=True):
    tiles, ops = meta["tiles"], meta["ops"]
    sb_first, sb_last = meta["sb_first"], meta["sb_last"]

    nc = bacc.Bacc("TRN2", target_bir_lowering=False, debug=False,
                   num_devices=NCORES, num_swdge_queues=2)

    xT_in = nc.dram_tensor("xT", [128, NPAD], F32, kind="ExternalInput").ap()
    W1_in = nc.dram_tensor("W1", [D, D], F32, kind="ExternalInput").ap()
    Wf1_in = nc.dram_tensor("Wf1", [RBF, D], F32, kind="ExternalInput").ap()
    Wf2_in = nc.dram_tensor("Wf2", [D, D], F32, kind="ExternalInput").ap()
    W2_in = nc.dram_tensor("W2", [D, D], F32, kind="ExternalInput").ap()
    b1bc_in = nc.dram_tensor("b1bc", [128, GRP * D], F32, kind="ExternalInput").ap()
    bf1c_in = nc.dram_tensor("bf1c", [128, 1], F32, kind="ExternalInput").ap()
    bf2bc_in = nc.dram_tensor("bf2bc", [128, GRP * D], F32, kind="ExternalInput").ap()
    b2bc_in = nc.dram_tensor("b2bc", [128, D], F32, kind="ExternalInput").ap()
    iota_in = nc.dram_tensor("iota512", [128, SB], F32, kind="ExternalInput").ap()
    idx_in = nc.dram_tensor("idx16", [128, T * 8], I16, kind="ExternalInput").ap()
    rl_in = nc.dram_tensor("rl", [128, T], F32, kind="ExternalInput").ap()
    rbfT_in = nc.dram_tensor("rbfT", [RBF, T * 128], F32, kind="ExternalInput").ap()
    out_ext = nc.dram_tensor("out", [NPC, D], F32, kind="ExternalOutput").ap()

    # xw table, row-major: node n at row n, features contiguous
    tab_h = nc.dram_tensor("xwtab", [NPAD, D], F32)
    tab = tab_h.ap()
    # per-quarter gather views: [QR, 128] starting at row q*QR
    tab_q = [bass.AP(tab.tensor, q * QR * D, [[D, QR], [1, D]])
             for q in range(NQ)]

    nc.gpsimd.load_library(library_config.mlp)

    NCH = (NTX + XCH - 1) // XCH

    with tile.TileContext(nc) as tc:
        with ExitStack() as ctx:
            res = ctx.enter_context(tc.tile_pool(name="res", bufs=1))
            xpool = ctx.enter_context(tc.tile_pool(name="xch", bufs=2))
            spool = ctx.enter_context(tc.tile_pool(name="stage", bufs=2))
            gpool = ctx.enter_context(tc.tile_pool(name="gath", bufs=4))
            rpool = ctx.enter_context(tc.tile_pool(name="rbfp", bufs=4))
            wpool = ctx.enter_context(tc.tile_pool(name="work", bufs=3))
            sspool = ctx.enter_context(tc.tile_pool(name="spool", bufs=2))
            hpool = ctx.enter_context(tc.tile_pool(name="hts", bufs=2))
            opool = ctx.enter_context(tc.tile_pool(name="outs", bufs=2))
            pp_mm = ctx.enter_context(tc.tile_pool(name="psmm", bufs=1, space="PSUM"))
            pp_h1 = ctx.enter_context(tc.tile_pool(name="psh1", bufs=1, space="PSUM"))
            pp_out = ctx.enter_context(tc.tile_pool(name="psout", bufs=GRPSB,
                                                    space="PSUM"))

            W1 = res.tile([D, D], F32); nc.sync.dma_start(W1[:], W1_in[:])
            Wf1 = res.tile([RBF, D], F32); nc.sync.dma_start(Wf1[:], Wf1_in[:])
            Wf2 = res.tile([D, D], F32); nc.sync.dma_start(Wf2[:], Wf2_in[:])
            W2 = res.tile([D, D], F32); nc.sync.dma_start(W2[:], W2_in[:])
            b1bc4 = res.tile([128, GRP * D], F32); nc.sync.dma_start(b1bc4[:], b1bc_in[:])
            bf1c = res.tile([128, 1], F32); nc.sync.dma_start(bf1c[:], bf1c_in[:])
            bf2bc = res.tile([128, GRP * D], F32); nc.sync.dma_start(bf2bc[:], bf2bc_in[:])
            b2bc = res.tile([128, D], F32); nc.sync.dma_start(b2bc[:], b2bc_in[:])
            iota = res.tile([128, SB], F32); nc.sync.dma_start(iota[:], iota_in[:])
            idx_sb = res.tile([128, T * 8], I16); nc.sync.dma_start(idx_sb[:], idx_in[:])
            rl = res.tile([128, T], F32); nc.sync.dma_start(rl[:], rl_in[:])

            for _rep in range(reps):
                # ---- phase A: xw = x @ W1 + b1 for all nodes ----
                for ch in range(NCH if do_phase_a else 0):
                    i0 = ch * XCH
                    nt = min(XCH, NTX - i0)
                    xch = xpool.tile([128, XCH * 128], F32, tag="xch")
                    nc.sync.dma_start(xch[:, :nt * 128],
                                      xT_in[:, i0 * 128:(i0 + nt) * 128])
                    stage = spool.tile([128, XCH * 128], F32, tag="stage")
                    for i0g in range(0, nt, GRP):
                        gg = min(GRP, nt - i0g)
                        xw_ps = pp_h1.tile([128, GRP * 128], F32, tag="h1",
                                           space="PSUM")
                        for i in range(i0g, i0g + gg):
                            nc.tensor.matmul(
                                xw_ps[:, (i - i0g) * 128:(i - i0g + 1) * 128],
                                lhsT=xch[:, i * 128:(i + 1) * 128],
                                rhs=W1[:], start=True, stop=True)
                        nc.vector.tensor_add(
                            stage[:, i0g * 128:(i0g + gg) * 128],
                            xw_ps[:, :gg * 128], b1bc4[:, :gg * 128])
                    # write [nt*128 rows, 128] of the table, partition-major
                    wv = bass.AP(tab.tensor, i0 * 128 * D,
                                 [[D, 128], [128 * D, nt], [1, D]])
                    nc.sync.dma_start(wv, stage[:, :nt * 128])

                # ---- phase B ----
                outT = {}
                for opi, (t0, ntq, qnum) in enumerate(ops if do_phase_b else []):
                    q = meta["op_quarter"][opi]
                    xwg = gpool.tile([128, OPT, 128], F32, tag="xwg")
                    if do_gather:
                        nc.gpsimd.dma_gather(
                            out_ap=xwg[:, :ntq, :],
                            in_ap=tab_q[q],
                            idxs_ap=idx_sb[:, t0 * 8:(t0 + ntq) * 8],
                            num_idxs=ntq * 128,
                            num_idxs_reg=ntq * 128,
                            elem_size=D,
                            queue_num=(qnum if qmode == "alt" else 0))
                    else:
                        nc.vector.memset(xwg[:, :ntq, :], 1.0)
                    rbft = rpool.tile([RBF, OPT * 128], F32, tag="rbf")
                    nc.sync.dma_start(
                        rbft[:, :ntq * 128],
                        rbfT_in[:, t0 * 128:(t0 + ntq) * 128])
                    S3 = sspool.tile([128, OPT, SB], F32, tag="S3")
                    iota_a = iota[:]
                    iota_v = bass.AP(iota_a.tensor, iota_a.offset,
                                     [iota_a.ap[0], [0, ntq], [1, SB]])
                    rl_s = rl[:, t0:t0 + ntq]
                    rl_v = bass.AP(rl_s.tensor, rl_s.offset,
                                   [rl_s.ap[0], [1, ntq], [0, SB]])
                    nc.vector.tensor_tensor(
                        out=S3[:, :ntq, :], in0=iota_v, in1=rl_v,
                        op=mybir.AluOpType.is_equal)
                    if not do_compute:
                        continue
                    for g0 in range(0, ntq, GRP):
                        gsz = min(GRP, ntq - g0)
                        h1 = pp_h1.tile([128, GRP * 128], F32, tag="h1",
                                        space="PSUM")
                        for c0 in range(0, gsz * 128, 512):
                            cw = min(512, gsz * 128 - c0)
                            nc.tensor.matmul(
                                h1[:, c0:c0 + cw], lhsT=Wf1[:],
                                rhs=rbft[:, g0 * 128 + c0:g0 * 128 + c0 + cw],
                                start=True, stop=True)
                        hT = hpool.tile([128, GRP * 128], F32, tag="hT")
                        nc.scalar.activation(
                            hT[:, :gsz * 128], h1[:, :gsz * 128],
                            mybir.ActivationFunctionType.Silu,
                            bias=bf1c[:, :1], scale=1.0)
                        filt_ps = pp_h1.tile([128, GRP * 128], F32,
                                             tag="h1", space="PSUM")
                        for i in range(gsz):
                            nc.tensor.matmul(
                                filt_ps[:, i * 128:(i + 1) * 128],
                                lhsT=hT[:, i * 128:(i + 1) * 128],
                                rhs=Wf2[:], start=True, stop=True)
                        msg = wpool.tile([128, GRP * 128], F32, tag="msg")
                        nc.vector.scalar_tensor_tensor(
                            out=msg[:, :gsz * 128], in0=filt_ps[:, :gsz * 128],
                            scalar=1.0, in1=bf2bc[:, :gsz * 128],
                            op0=mybir.AluOpType.mult,
                            op1=mybir.AluOpType.add)
                        msgm = wpool.tile([128, GRP, 128], F32, tag="msgm")
                        nc.vector.tensor_mul(msgm[:, :gsz, :],
                                             msg[:, :gsz * 128],
                                             xwg[:, g0:g0 + gsz, :])
                        for i in range(gsz):
                            t = t0 + g0 + i
                            s = tiles[t]
                            if do_scatter and s not in outT:
                                outT[s] = pp_out.tile([128, SB], F32,
                                                      tag="outT", space="PSUM",
                                                      name=f"oT{s}")
                            if do_scatter:
                                nc.tensor.matmul(
                                    outT[s][:], lhsT=msgm[:, i, :],
                                    rhs=S3[:, g0 + i, :],
                                    start=(t == sb_first[s]),
                                    stop=(t == sb_last[s]))
                            if do_scatter and t == sb_last[s]:
                                # retire superblock s
                                oT = outT.pop(s)
                                outT_sb = opool.tile([128, SB], F32,
                                                     tag="outTsb")
                                nc.vector.tensor_copy(outT_sb[:], oT[:])
                                r0 = s * SB
                                rows = min(SB, NPC - r0)
                                nchunk = (rows + 127) // 128
                                for cc in range(nchunk):
                                    fin_ps = pp_mm.tile([128, 128], F32,
                                                        tag="mm128",
                                                        space="PSUM")
                                    nc.tensor.matmul(
                                        fin_ps[:],
                                        lhsT=outT_sb[:, cc * 128:(cc + 1) * 128],
                                        rhs=W2[:], start=True, stop=True)
                                    fin = opool.tile([128, 128], F32,
                                                     tag="fin_sb")
                                    nc.vector.tensor_add(fin[:], fin_ps[:],
                                                         b2bc[:])
                                    rr = min(128, rows - cc * 128)
                                    nc.sync.dma_start(
                                        out_ext[r0 + cc * 128:
                                                r0 + cc * 128 + rr, :],
                                        fin[:rr, :])
    nc.compile()
    return nc


def _make_in_maps(x, edge_index, edge_rbf, W1, b1, Wf1, bf1, Wf2, bf2, W2, b2,
                  cores, T):
    xT = np.zeros((128, NPAD), dtype=np.float32)
    xp = np.zeros((NPAD, D), dtype=np.float32)
    xp[:N_NODES] = np.asarray(x, dtype=np.float32)
    # xT[:, i*128:(i+1)*128] is node-tile i, feature-on-partition
    xT[:] = xp.reshape(NTX, 128, D).transpose(2, 0, 1).reshape(D, NPAD)

    common = {
        "xT": xT,
        "W1": np.asarray(W1, np.float32),
        "Wf1": np.asarray(Wf1, np.float32),
        "Wf2": np.asarray(Wf2, np.float32),
        "W2": np.asarray(W2, np.float32),
        "b1bc": np.tile(np.broadcast_to(np.asarray(b1, np.float32), (128, D)), (1, GRP)).copy(),
        "bf1c": np.asarray(bf1, np.float32).reshape(128, 1).copy(),
        "bf2bc": np.tile(np.broadcast_to(np.asarray(bf2, np.float32), (128, D)), (1, GRP)).copy(),
        "b2bc": np.broadcast_to(np.asarray(b2, np.float32), (128, D)).copy(),
        "iota512": np.broadcast_to(np.arange(SB, dtype=np.float32),
                                   (128, SB)).copy(),
    }
    in_maps = []
    for c in range(NCORES):
        m = dict(common)
        m["idx16"] = cores[c]["idx16"]
        m["rl"] = cores[c]["rl"]
        m["rbfT"] = cores[c]["rbfT"]
        in_maps.append(m)
    return in_maps


_CACHE = {}


def kernel(x, edge_index, edge_rbf, W1, b1, Wf1, bf1, Wf2, bf2, W2, b2):
    cores, tpbq, T, meta = _preprocess(edge_index, edge_rbf)
    key = (T, tuple(np.asarray(tpbq).ravel().tolist()))
    if key not in _CACHE:
        _CACHE[key] = _build_program(T, meta, reps=1)
    nc = _CACHE[key]
    in_maps = _make_in_maps(x, edge_index, edge_rbf, W1, b1, Wf1, bf1, Wf2,
                            bf2, W2, b2, cores, T)
    res = run_bass_kernel_spmd(nc, in_maps, list(range(NCORES)))
    out = np.concatenate([res.results[c]["out"] for c in range(NCORES)],
                         axis=0)
    return out.astype(np.float32)
